# revision 25
# baseline (speedup 1.0000x reference)
"""Trainium2 Bass kernel for nn_MEMOIR (scatter_memory).

Math: out = x @ (weight + gate*final_mask (.) new_weight).T where
  f = mean(x[0, :pb+1, :]) over s; mask0 = topk_set(|f|, 2048) in j-space;
  overlap[n] = sum_j mask0[j] * stored_masks[n, perm[j]];
  gate = max(overlap)/2048 >= 0.6; final_mask = stored_masks[argmax].

Sharding: 8 cores on a 2x4 grid — 2 shards of B*S rows x 4 shards of O cols.
Each core computes out_shard[4096, 1024] = xT_shard.T @ wT_shard with fp32r
matmuls (full PE rate, ~19-bit mantissa) and a fully resident weight shard.

The mask/gate computation is replicated on every core and is deliberately
PE-free (DVE + ScalarE + GpSimd only): the TensorE instruction stream is
in-order, so any mask-phase PE op would stall the main matmul stream behind
the slow serial mask chain. Threshold selection uses 30 bisection steps with
gpsimd partition_all_reduce; the permutation is applied with a gpsimd
ap_gather of the stored-mask rows (the scatter_memory pattern); the mask
broadcast uses gpsimd partition_broadcast.

The gated residual correction (gate is False for in-distribution inputs)
runs as a second kernel launched by the host only when the device-computed
gate fires; the gate/argmax one-hot is read back via a small aux output.
"""

import ml_dtypes
import numpy as np

import concourse.bacc as bacc
import concourse.bass_isa as bass_isa
import concourse.mybir as mybir
import concourse.tile as tile
from concourse.bass_utils import run_bass_kernel_spmd

F32 = mybir.dt.float32
F32R = mybir.dt.float32r
BF16 = mybir.dt.bfloat16
I16 = mybir.dt.int16

B, S, D, O = 4, 2048, 4096, 4096
BS = B * S
RG, CG = 2, 4  # grid: RG shards over B*S rows, CG shards over O cols
MSH = BS // RG  # 4096 rows per core
OSH = O // CG  # 1024 out cols per core
KT = D // 128  # 32 k-tiles
NM = 100  # stored masks
PBMAX = 1025  # max prompt slice length (pb + 1, pb <= 1024)
TOPK = 2048
GATE_THRESH = 1228.5  # count >= 1229  <=>  overlap >= 0.6*2048 = 1228.8
BISECT_ITERS = 30

_CACHE = {}


def _build_main(do_mask=True, do_main=True):
    nc = bacc.Bacc("TRN2", target_bir_lowering=False, debug=False, num_devices=8)

    xT_d = nc.dram_tensor("xT", [D, MSH], F32R, kind="ExternalInput").ap()
    wT_d = nc.dram_tensor("wT", [D, OSH], F32R, kind="ExternalInput").ap()
    x0pT_d = nc.dram_tensor("x0pT", [D, PBMAX], F32R, kind="ExternalInput").ap()
    x0n_d = nc.dram_tensor("x0n", [PBMAX, D], F32, kind="ExternalInput").ap()
    psc_d = nc.dram_tensor("pscale", [128, 1], F32, kind="ExternalInput").ap()
    smn_d = nc.dram_tensor("smn_pad", [128, D], F32, kind="ExternalInput").ap()
    perm_d = nc.dram_tensor("perm16", [128, 256], I16, kind="ExternalInput").ap()
    iota_d = nc.dram_tensor("iota128", [128, 1], F32, kind="ExternalInput").ap()

    out_d = nc.dram_tensor("out", [MSH, OSH], F32, kind="ExternalOutput").ap()
    aux_d = nc.dram_tensor("aux", [128, 8], F32, kind="ExternalOutput").ap()

    with tile.TileContext(nc) as tc:
        with (
            tc.tile_pool(name="consts", bufs=1) as cpool,
            tc.tile_pool(name="wres", bufs=1) as wpool,
            tc.tile_pool(name="xstrip", bufs=2) as xpool,
            tc.tile_pool(name="outs", bufs=2) as opool,
            tc.tile_pool(name="mask", bufs=1) as mpool,
            tc.tile_pool(name="x0stage", bufs=1) as x0pool,
            tc.tile_pool(name="mainps", bufs=6, space="PSUM") as mainps,
        ):
            # ---- constants ----
            iota = cpool.tile([128, 1], F32, tag="iota")
            psc = cpool.tile([128, 1], F32, tag="psc")
            perm16 = cpool.tile([128, 256], I16, tag="perm16")
            nc.sync.dma_start(out=iota[:], in_=iota_d)
            nc.sync.dma_start(out=psc[:], in_=psc_d)
            nc.sync.dma_start(out=perm16[:], in_=perm_d)

            # ================= main matmul (pure PE stream) =================
            if do_main:
                xv = xT_d.rearrange("(k p) b -> p k b", p=128)
                wres = wpool.tile([128, KT * OSH], F32R, tag="wres")
                for h in range(OSH // 512):
                    for k in range(KT):
                        nc.sync.dma_start(
                            out=wres[:, k * OSH + h * 512 : k * OSH + h * 512 + 512],
                            in_=wT_d[k * 128 : (k + 1) * 128,
                                     h * 512 : (h + 1) * 512],
                        )
                for m in range(MSH // 128):
                    xs = xpool.tile([128, KT * 128], F32R, tag="xs")
                    xs3 = xs.rearrange("p (k b) -> p k b", k=KT)
                    for kc in range(8):
                        nc.sync.dma_start(
                            out=xs3[:, kc * 4 : (kc + 1) * 4, :],
                            in_=xv[:, kc * 4 : (kc + 1) * 4, m * 128 : (m + 1) * 128],
                        )
                    for n2 in range(OSH // 512):
                        ps = mainps.tile([128, 512], F32, tag="ps")
                        for k in range(KT):
                            nc.tensor.matmul(
                                ps[:],
                                lhsT=xs[:, k * 128 : (k + 1) * 128],
                                rhs=wres[:, k * OSH + n2 * 512 : k * OSH + n2 * 512 + 512],
                                start=(k == 0), stop=(k == KT - 1),
                            )
                        outt = opool.tile([128, 512], F32, tag="outt")
                        nc.vector.tensor_copy(out=outt[:], in_=ps[:])
                        nc.sync.dma_start(
                            out=out_d[m * 128 : (m + 1) * 128,
                                      n2 * 512 : (n2 + 1) * 512],
                            in_=outt[:],
                        )

            # ============ mask / gate phase (no TensorE instructions) ============
            if do_mask:
                # --- af32[p, k] = |mean_s x[0, s, k*128+p]| from transposed x0 ---
                af32 = mpool.tile([128, KT], F32R, tag="af32")
                for k in range(KT):
                    x0t = x0pool.tile([128, PBMAX], F32R, tag="x0t")
                    nc.gpsimd.dma_start(
                        out=x0t[:], in_=x0pT_d[k * 128 : (k + 1) * 128, :]
                    )
                    fsum = mpool.tile([128, 1], F32, tag="fsum")
                    nc.vector.tensor_reduce(
                        out=fsum[:], in_=x0t[:], axis=mybir.AxisListType.X,
                        op=mybir.AluOpType.add,
                    )
                    nc.scalar.activation(
                        af32[:, k : k + 1], fsum[:],
                        mybir.ActivationFunctionType.Abs, scale=psc[:],
                    )

                # --- global max -> every partition ---
                rpp = mpool.tile([128, 1], F32, tag="rpp")
                nc.vector.tensor_reduce(
                    out=rpp[:], in_=af32[:], axis=mybir.AxisListType.X,
                    op=mybir.AluOpType.max,
                )
                rbc = mpool.tile([128, 1], F32, tag="rbc")
                nc.gpsimd.partition_all_reduce(
                    out_ap=rbc[:], in_ap=rpp[:], channels=128,
                    reduce_op=bass_isa.ReduceOp.max,
                )

                # --- bisection for the TOPK-th |f| threshold ---
                tlo = mpool.tile([128, 1], F32, tag="tlo")
                width = mpool.tile([128, 1], F32, tag="width")
                tmid = mpool.tile([128, 1], F32, tag="tmid")
                cpart = mpool.tile([128, 1], F32, tag="cpart")
                cbc = mpool.tile([128, 1], F32, tag="cbc")
                qsel = mpool.tile([128, 1], F32, tag="qsel")
                junk32 = mpool.tile([128, KT], F32, tag="junk32")
                nc.vector.memset(tlo[:], 0.0)
                nc.vector.tensor_copy(out=width[:], in_=rbc[:])
                for it in range(BISECT_ITERS):
                    nc.vector.tensor_scalar_mul(out=width[:], in0=width[:], scalar1=0.5)
                    nc.vector.tensor_add(out=tmid[:], in0=tlo[:], in1=width[:])
                    nc.vector.tensor_scalar(
                        out=junk32[:], in0=af32[:], scalar1=tmid[:], scalar2=None,
                        op0=mybir.AluOpType.is_ge, op1=mybir.AluOpType.add,
                        accum_out=cpart[:],
                    )
                    nc.gpsimd.partition_all_reduce(
                        out_ap=cbc[:], in_ap=cpart[:], channels=128,
                        reduce_op=bass_isa.ReduceOp.add,
                    )
                    nc.vector.tensor_scalar(
                        out=qsel[:], in0=cbc[:], scalar1=float(TOPK), scalar2=None,
                        op0=mybir.AluOpType.is_ge,
                    )
                    # tlo += qsel * width
                    nc.vector.scalar_tensor_tensor(
                        out=tlo[:], in0=qsel[:], scalar=width[:], in1=tlo[:],
                        op0=mybir.AluOpType.mult, op1=mybir.AluOpType.add,
                    )

                # --- af_row[0, d] = |mean_s x[0, s, d]| from natural x0 ---
                # (independent fp32 recomputation; boundary disagreements with
                # af32 shift the overlap count by O(1) and are gate-safe)
                af_row = mpool.tile([1, D], F32, tag="big16")
                for dh in range(2):
                    acc = mpool.tile([128, D // 2], F32, tag="buf8k")
                    nc.vector.memset(acc[:], 0.0)
                    for st in range(9):
                        rows = 128 if st < 8 else PBMAX - 8 * 128
                        x0s = x0pool.tile([128, D // 2], F32, tag="x0t")
                        nc.sync.dma_start(
                            out=x0s[:rows, :],
                            in_=x0n_d[st * 128 : st * 128 + rows,
                                      dh * (D // 2) : (dh + 1) * (D // 2)],
                        )
                        nc.vector.tensor_add(
                            out=acc[:rows, :], in0=acc[:rows, :], in1=x0s[:rows, :]
                        )
                    nc.gpsimd.partition_all_reduce(
                        out_ap=acc[:], in_ap=acc[:], channels=128,
                        reduce_op=bass_isa.ReduceOp.add,
                    )
                    nc.scalar.activation(
                        af_row[:, dh * (D // 2) : (dh + 1) * (D // 2)],
                        acc[0:1, :],
                        mybir.ActivationFunctionType.Abs, scale=psc[0:1, :],
                    )
                # mask row (bf16 0/1) and its per-quarter partition broadcast
                m0row = mpool.tile([1, D], BF16, tag="buf8k")
                nc.vector.tensor_scalar(
                    out=m0row[:], in0=af_row[:], scalar1=tlo[0:1, :], scalar2=None,
                    op0=mybir.AluOpType.is_ge,
                )

                # --- permuted overlap, stored-mask index n on partitions ---
                smn = mpool.tile([128, D], F32, tag="big16")
                nc.gpsimd.dma_start(out=smn[:], in_=smn_d)
                m0bc = mpool.tile([128, 1024], BF16, tag="m0bc")
                ov4 = mpool.tile([128, 4], F32, tag="ov4")
                for q in range(4):
                    smp = x0pool.tile([128, 1024], F32, tag="x0t")
                    nc.gpsimd.ap_gather(
                        out_ap=smp.rearrange("p (j d) -> p j d", d=1),
                        in_ap=smn.rearrange("p (j d) -> p j d", d=1),
                        idxs_ap=perm16[:, q * 64 : (q + 1) * 64],
                        channels=128, num_elems=D, d=1, num_idxs=1024,
                    )
                    nc.gpsimd.partition_broadcast(
                        out_ap=m0bc[:], in_ap=m0row[:, q * 1024 : (q + 1) * 1024],
                        channels=128,
                    )
                    nc.vector.scalar_tensor_tensor(
                        out=smp[:], in0=smp[:], scalar=1.0, in1=m0bc[:],
                        op0=mybir.AluOpType.mult, op1=mybir.AluOpType.mult,
                        accum_out=ov4[:, q : q + 1],
                    )
                ov = mpool.tile([128, 1], F32, tag="ov")
                nc.vector.tensor_reduce(
                    out=ov[:], in_=ov4[:], axis=mybir.AxisListType.X,
                    op=mybir.AluOpType.add,
                )
                # tie-broken score tb[n] = count - n/256 ; argmax = first max
                tb = mpool.tile([128, 1], F32, tag="tb")
                nc.vector.scalar_tensor_tensor(
                    out=tb[:], in0=iota[:], scalar=-1.0 / 256.0, in1=ov[:],
                    op0=mybir.AluOpType.mult, op1=mybir.AluOpType.add,
                )
                tbmax = mpool.tile([128, 1], F32, tag="tbmax")
                nc.gpsimd.partition_all_reduce(
                    out_ap=tbmax[:], in_ap=tb[:], channels=128,
                    reduce_op=bass_isa.ReduceOp.max,
                )
                gate = mpool.tile([128, 1], F32, tag="gate")
                nc.vector.tensor_scalar(
                    out=gate[:], in0=tbmax[:], scalar1=GATE_THRESH, scalar2=None,
                    op0=mybir.AluOpType.is_ge,
                )
                onehot = mpool.tile([128, 1], F32, tag="onehot")
                nc.vector.tensor_scalar(
                    out=onehot[:], in0=tb[:], scalar1=tbmax[:], scalar2=gate[:],
                    op0=mybir.AluOpType.is_ge, op1=mybir.AluOpType.mult,
                )
                aux_sb = mpool.tile([128, 8], F32, tag="aux_sb")
                nc.vector.memset(aux_sb[:], 0.0)
                nc.vector.tensor_copy(out=aux_sb[:, 0:1], in_=gate[:])
                nc.vector.tensor_copy(out=aux_sb[:, 1:2], in_=tbmax[:])
                nc.vector.tensor_copy(out=aux_sb[:, 2:3], in_=tlo[:])
                nc.vector.tensor_copy(out=aux_sb[:, 3:4], in_=cbc[:])
                nc.vector.tensor_copy(out=aux_sb[:, 4:5], in_=onehot[:])
                nc.vector.tensor_copy(out=aux_sb[:, 5:6], in_=ov[:])
                nc.sync.dma_start(out=aux_d, in_=aux_sb[:])

    nc.compile()
    return nc


def _build_correction():
    """out += xT.T @ (fm (.) nwT).T — only launched when the gate fired."""
    nc = bacc.Bacc("TRN2", target_bir_lowering=False, debug=False, num_devices=8)
    xT_d = nc.dram_tensor("xT", [D, MSH], F32R, kind="ExternalInput").ap()
    nwT_d = nc.dram_tensor("nwT", [D, OSH], F32R, kind="ExternalInput").ap()
    smn_d = nc.dram_tensor("smn", [NM, D], BF16, kind="ExternalInput").ap()
    oh_d = nc.dram_tensor("onehot", [NM, 1], BF16, kind="ExternalInput").ap()
    outp_d = nc.dram_tensor("outp", [MSH, OSH], F32, kind="ExternalInput").ap()
    out_d = nc.dram_tensor("out", [MSH, OSH], F32, kind="ExternalOutput").ap()

    with tile.TileContext(nc) as tc:
        with (
            tc.tile_pool(name="fm", bufs=1) as fpool,
            tc.tile_pool(name="wres", bufs=1) as wpool,
            tc.tile_pool(name="xstrip", bufs=2) as xpool,
            tc.tile_pool(name="outs", bufs=2) as opool,
            tc.tile_pool(name="fps", bufs=1, space="PSUM") as fps,
            tc.tile_pool(name="mainps", bufs=4, space="PSUM") as mainps,
        ):
            # fm[p, k] = stored_masks[best, k*128+p] = sum_n smn[n, .] * onehot[n]
            smn = fpool.tile([NM, D], BF16, tag="smn")
            nc.gpsimd.dma_start(out=smn[:], in_=smn_d)
            ohc = fpool.tile([NM, 1], BF16, tag="ohc")
            nc.sync.dma_start(out=ohc[:], in_=oh_d)
            fmps = fps.tile([128, KT], F32, tag="fmps")
            for k in range(KT):
                nc.tensor.matmul(
                    fmps[:, k : k + 1], lhsT=smn[:, k * 128 : (k + 1) * 128],
                    rhs=ohc[:], start=True, stop=True,
                )
            fm = fpool.tile([128, KT], F32, tag="fm")
            nc.vector.tensor_copy(out=fm[:], in_=fmps[:])

            xv = xT_d.rearrange("(k p) b -> p k b", p=128)
            nwres = wpool.tile([128, KT * OSH], F32R, tag="nwres")
            for k in range(KT):
                nc.sync.dma_start(
                    out=nwres[:, k * OSH : (k + 1) * OSH],
                    in_=nwT_d[k * 128 : (k + 1) * 128, :],
                )
            # apply fm along d (partition scalar per k-tile)
            for k in range(KT):
                nc.vector.tensor_scalar(
                    out=nwres[:, k * OSH : (k + 1) * OSH],
                    in0=nwres[:, k * OSH : (k + 1) * OSH],
                    scalar1=fm[:, k : k + 1], scalar2=None,
                    op0=mybir.AluOpType.mult,
                )
            for m in range(MSH // 128):
                xs = xpool.tile([128, KT * 128], F32R, tag="xs")
                nc.sync.dma_start(
                    out=xs.rearrange("p (k b) -> p k b", k=KT),
                    in_=xv[:, :, m * 128 : (m + 1) * 128],
                )
                prev = opool.tile([128, OSH], F32, tag="prev")
                nc.sync.dma_start(
                    out=prev[:], in_=outp_d[m * 128 : (m + 1) * 128, :]
                )
                outt = opool.tile([128, OSH], F32, tag="outt")
                for n2 in range(OSH // 512):
                    ps = mainps.tile([128, 512], F32, tag="ps")
                    for k in range(KT):
                        nc.tensor.matmul(
                            ps[:],
                            lhsT=xs[:, k * 128 : (k + 1) * 128],
                            rhs=nwres[:, k * OSH + n2 * 512 : k * OSH + n2 * 512 + 512],
                            start=(k == 0), stop=(k == KT - 1),
                        )
                    nc.vector.tensor_tensor(
                        out=outt[:, n2 * 512 : (n2 + 1) * 512],
                        in0=ps[:], in1=prev[:, n2 * 512 : (n2 + 1) * 512],
                        op=mybir.AluOpType.add,
                    )
                nc.sync.dma_start(
                    out=out_d[m * 128 : (m + 1) * 128, :], in_=outt[:]
                )
    nc.compile()
    return nc


def _prep_in_maps(x, weight, new_weight, permutation, stored_masks, pb):
    f32 = np.float32
    x = np.asarray(x)
    weight = np.asarray(weight)
    stored_masks = np.asarray(stored_masks)
    permutation = np.asarray(permutation)
    x2 = np.ascontiguousarray(x.reshape(BS, D), dtype=f32)
    pb1 = int(pb) + 1
    x0pT = np.zeros((D, PBMAX), dtype=f32)
    x0pT[:, :pb1] = x2[:pb1, :].T
    x0n = np.zeros((PBMAX, D), dtype=f32)
    x0n[:pb1, :] = x2[:pb1, :]
    psc = np.full((128, 1), 1.0 / pb1, dtype=f32)
    smn_pad = np.zeros((128, D), dtype=f32)
    smn_pad[:NM, :] = stored_masks.astype(f32)
    p16 = np.asarray(permutation, dtype=np.int16)
    perm16 = np.concatenate(
        [np.tile(p16[q * 1024 : (q + 1) * 1024].reshape(64, 16).T, (8, 1))
         for q in range(4)],
        axis=1,
    ).copy()
    shared = {
        "x0pT": x0pT,
        "x0n": x0n,
        "pscale": psc,
        "smn_pad": smn_pad,
        "perm16": perm16,
        "iota128": np.arange(128, dtype=f32).reshape(128, 1),
    }
    xTs = [
        np.ascontiguousarray(x2[r * MSH : (r + 1) * MSH, :].T) for r in range(RG)
    ]
    wTs = [
        np.ascontiguousarray(weight[g * OSH : (g + 1) * OSH, :].T.astype(f32))
        for g in range(CG)
    ]
    in_maps = []
    for c in range(8):
        r, g = divmod(c, CG)
        in_maps.append({"xT": xTs[r], "wT": wTs[g], **shared})
    return in_maps, xTs


def kernel(x, weight, new_weight, permutation, stored_masks, prompt_boundary):
    x = np.asarray(x)
    weight = np.asarray(weight)
    new_weight = np.asarray(new_weight)
    permutation = np.asarray(permutation)
    stored_masks = np.asarray(stored_masks)
    prompt_boundary = int(np.asarray(prompt_boundary))
    if "main" not in _CACHE:
        _CACHE["main"] = _build_main()
    nc = _CACHE["main"]

    in_maps, xTs = _prep_in_maps(
        x, weight, new_weight, permutation, stored_masks, prompt_boundary
    )
    res = run_bass_kernel_spmd(nc, in_maps, list(range(8)))

    out = np.empty((BS, O), dtype=np.float32)
    for c in range(8):
        r, g = divmod(c, CG)
        out[r * MSH : (r + 1) * MSH, g * OSH : (g + 1) * OSH] = res.results[c]["out"]

    aux = res.results[0]["aux"]
    if aux[0, 0] != 0.0:
        # residual memory is on: out += x_hashed @ new_weight.T
        if "corr" not in _CACHE:
            _CACHE["corr"] = _build_correction()
        ncc = _CACHE["corr"]
        onehot = np.ascontiguousarray(aux[:NM, 4].reshape(NM, 1)).astype(
            ml_dtypes.bfloat16
        )
        smn = np.ascontiguousarray(stored_masks.astype(ml_dtypes.bfloat16))
        nwTs = [
            np.ascontiguousarray(
                new_weight[g * OSH : (g + 1) * OSH, :].T.astype(np.float32)
            )
            for g in range(CG)
        ]
        in_maps2 = []
        for c in range(8):
            r, g = divmod(c, CG)
            in_maps2.append(
                {
                    "xT": xTs[r],
                    "nwT": nwTs[g],
                    "smn": smn,
                    "onehot": onehot,
                    "outp": np.ascontiguousarray(
                        out[r * MSH : (r + 1) * MSH, g * OSH : (g + 1) * OSH]
                    ),
                }
            )
        res2 = run_bass_kernel_spmd(ncc, in_maps2, list(range(8)))
        for c in range(8):
            r, g = divmod(c, CG)
            out[r * MSH : (r + 1) * MSH, g * OSH : (g + 1) * OSH] = res2.results[c][
                "out"
            ]

    return out.reshape(B, S, O)
